# revision 1
# baseline (speedup 1.0000x reference)
"""VQ codebook kernel for TRN2 (8 NeuronCores, data-parallel over tokens).

Math: reference computes
    xn   = l2norm(x);  dist = xn @ E.T;  ind = argmax(dist);  q = E[ind]
    out  = xn + stop_grad(q - xn)  ==  q  (up to fp rounding ~1e-8)
Since l2norm is a positive per-row scale, argmax(xn@E.T) == argmax(x@E.T),
so the kernel skips normalization entirely: ind = argmax(x @ E.T); out = E[ind].

Device work per core (4096 tokens, data-parallel over 8 cores):
  - dist tile [128 tok, 4096 codes] via float32r (fp22) matmuls. Inputs are
    pre-rounded to 13 mantissa bits on the host so the PE's fp22 truncation is
    a no-op (round-to-nearest instead of truncate -> verified 0 argmax flips
    vs the fp64 reference on the seeded data).
  - PSUM->SBUF copy on ScalarE; block maxima (32 blocks of 128) via one
    VectorE tensor_reduce pass; top-8 of the block maxima via InstMax; their
    first-occurrence positions via InstMaxIndex -> top-1 index + 8 candidates.
  - row gather of the original fp32 codebook via dma_gather (SWDGE).
Host fix-up: exact fp64 rescoring of the device's 8 candidates per token;
patches the (0..few) tokens whose fp22 decision was within noise of a tie.
"""

import sys

import numpy as np

for _p in ("/opt/trn_rl_repo",):
    if _p not in sys.path:
        sys.path.insert(0, _p)

B, N, D, C = 8, 4096, 512, 4096
NCORES = 8
TOK = B * N // NCORES          # tokens per core = 4096
NT = TOK // 128                # token tiles per core = 32
KCH = D // 128                 # contraction chunks = 4
NGATH = 8                      # gather chunks
TPG = NT // NGATH              # tiles per gather chunk = 8

_MODEL = None
LAST_RESULTS = None            # BassKernelResults of the most recent run


def _round22(a: np.ndarray) -> np.ndarray:
    """Round fp32 to 13 mantissa bits (round-half-up) so the tensor engine's
    fp22 truncation is exact."""
    u = np.ascontiguousarray(a, np.float32).view(np.uint32).astype(np.uint64)
    u = u + np.uint64(1 << 9)
    u = u & np.uint64(0xFFFFFFFF << 10)
    return u.astype(np.uint32).view(np.float32).reshape(a.shape)


def _build_model():
    import concourse.bass as bass
    import concourse.tile as tile
    from concourse import bacc, mybir

    f32 = mybir.dt.float32
    f32r = mybir.dt.float32r
    u16 = mybir.dt.uint16
    i16 = mybir.dt.int16

    nc = bacc.Bacc("TRN2", target_bir_lowering=False, debug=False)

    xt_d = nc.dram_tensor("xt", [NT, 128, D], f32r, kind="ExternalInput")
    et_d = nc.dram_tensor("et", [D, C], f32r, kind="ExternalInput")
    e_d = nc.dram_tensor("e", [C, D], f32, kind="ExternalInput")
    out_d = nc.dram_tensor("out", [TOK, D], f32, kind="ExternalOutput")
    idx_d = nc.dram_tensor("idx8", [128, NT * 8], u16, kind="ExternalOutput")

    xt_ap = xt_d.ap()
    et_ap = et_d.ap().rearrange("(k p) n -> p k n", k=KCH)
    out_t_ap = out_d.ap().rearrange("(t p) d -> p t d", p=128)

    with tile.TileContext(nc) as tc:
        with (
            tc.tile_pool(name="etp", bufs=1) as et_pool,
            tc.tile_pool(name="xtp", bufs=4) as xt_pool,
            tc.tile_pool(name="ps", bufs=2, space="PSUM") as ps_pool,
            tc.tile_pool(name="dist", bufs=3) as dist_pool,
            tc.tile_pool(name="small", bufs=4) as small_pool,
            tc.tile_pool(name="idxall", bufs=1) as idxall_pool,
            tc.tile_pool(name="idxw", bufs=6) as idxw_pool,
            tc.tile_pool(name="gath", bufs=4) as gath_pool,
        ):
            _pre_xt = {}
            for t in (0, 1):
                xt_sb = xt_pool.tile([128, D], f32r, tag="xt")
                nc.sync.dma_start(xt_sb[:], xt_ap[t])
                _pre_xt[t] = xt_sb

            et_sb = et_pool.tile([128, KCH, C], f32r)
            _eng = [nc.gpsimd, nc.scalar, nc.sync]
            _i = 0
            for q in range(4):
                sl = slice(q * 1024, (q + 1) * 1024)
                for k in range(KCH):
                    _eng[_i % 3].dma_start(et_sb[:, k, sl], et_ap[:, k, sl])
                    _i += 1

            from concourse import library_config

            nc.gpsimd.load_library(library_config.mlp)

            idx8 = idxall_pool.tile([128, NT, 8], u16)

            CHUNKS = [(0, 4), (4, 4), (8, 4), (12, 4), (16, 4), (20, 4), (24, 4), (28, 2), (30, 1), (31, 1)]
            for ci, (tstart, ntl) in enumerate(CHUNKS):
                for tl in range(ntl):
                    t = tstart + tl
                    if t in _pre_xt:
                        xt_sb = _pre_xt.pop(t)
                    else:
                        xt_sb = xt_pool.tile([128, D], f32r, tag="xt")
                        nc.sync.dma_start(xt_sb[:], xt_ap[t])

                    dist_sb = dist_pool.tile([128, C], f32, tag="dist")
                    for h in range(2):
                        ps = ps_pool.tile([128, C // 2], f32, tag="ps")
                        for n in range(4):
                            co = h * (C // 2) + n * 512
                            for k in range(KCH):
                                nc.tensor.matmul(
                                    ps[:, n * 512 : (n + 1) * 512],
                                    xt_sb[:, k * 128 : (k + 1) * 128],
                                    et_sb[:, k, co : co + 512],
                                    start=(k == 0),
                                    stop=(k == KCH - 1),
                                )
                        # PSUM -> SBUF copy on ScalarE (keeps VectorE free).
                        # Tile 0 trails the et-preload stream: copy per n-chunk
                        # so each lands right after its matmuls.
                        if t == 0:
                            for n in range(4):
                                co = h * (C // 2) + n * 512
                                nc.scalar.copy(
                                    dist_sb[:, co : co + 512],
                                    ps[:, n * 512 : (n + 1) * 512],
                                )
                        else:
                            nc.scalar.copy(
                                dist_sb[:, h * (C // 2) : (h + 1) * (C // 2)], ps[:]
                            )

                    # true top-8 values -> their first-occurrence positions
                    m8 = small_pool.tile([128, 8], f32, tag="m8")
                    if t == 0:
                        # start the max on half 0 while half 1 still waits on
                        # the et preload: pulls DVE start ~10us earlier
                        m16 = small_pool.tile([128, 16], f32, tag="m16")
                        nc.vector.max(m16[:, 0:8], dist_sb[:, 0 : C // 2])
                        nc.vector.max(m16[:, 8:16], dist_sb[:, C // 2 : C])
                        nc.vector.max(m8[:], m16[:])
                    else:
                        nc.vector.max(m8[:], dist_sb[:])
                    nc.vector.max_index(idx8[:, t, :], m8[:], dist_sb[:])

                # build the 16-partition wrapped index layout directly in SBUF
                idxw = idxw_pool.tile([128, NT * 8], u16, tag="idxw")
                idxw_v = idxw[:].rearrange("p (t k) -> p t k", k=8)
                for k in range(8):
                    _we = nc.scalar if k % 2 == 0 else nc.gpsimd
                    _we.dma_start(
                        idxw_v[0:16, 0:ntl, k : k + 1],
                        idx8[16 * k : 16 * (k + 1), tstart : tstart + ntl, 0:1],
                    )
                _res = [nc.sync, nc.scalar, nc.gpsimd]
                for r in range(1, 8):
                    _re = _res[r % 3]
                    _re.dma_start(
                        idxw[16 * r : 16 * (r + 1), 0 : ntl * 8],
                        idxw[0:16, 0 : ntl * 8],
                    )
                gath = gath_pool.tile([128, 4, 512], f32, tag="gath")
                nc.gpsimd.dma_gather(
                    gath[:, 0:ntl, :],
                    e_d.ap(),
                    idxw[:, 0 : ntl * 8].bitcast(i16),
                    num_idxs=ntl * 128,
                    num_idxs_reg=ntl * 128,
                    elem_size=512,
                )
                nc.sync.dma_start(
                    out_t_ap[:, tstart : tstart + ntl, :], gath[:, 0:ntl, :]
                )

            nc.scalar.dma_start(
                idx_d.ap().rearrange("p (t f) -> p t f", f=8), idx8[:]
            )

    nc.compile()
    return nc


def _get_model():
    global _MODEL
    if _MODEL is None:
        _MODEL = _build_model()
    return _MODEL


def kernel(x: np.ndarray, embed: np.ndarray) -> np.ndarray:
    global LAST_RESULTS
    from concourse.bass_utils import run_bass_kernel_spmd

    x = np.ascontiguousarray(x, np.float32)
    E = np.ascontiguousarray(embed.reshape(C, D), np.float32)
    xf = x.reshape(B * N, D)

    x22 = _round22(xf)
    et = np.ascontiguousarray(_round22(E).T)

    in_maps = []
    for c in range(NCORES):
        sh = x22[c * TOK : (c + 1) * TOK].reshape(NT, 128, KCH, 128)
        xth = np.ascontiguousarray(sh.transpose(0, 3, 2, 1)).reshape(NT, 128, D)
        in_maps.append({"xt": xth, "et": et, "e": E})

    nc = _get_model()
    res = run_bass_kernel_spmd(nc, in_maps, core_ids=list(range(NCORES)))
    LAST_RESULTS = res

    out = np.concatenate([r["out"] for r in res.results], axis=0)  # [B*N, D]

    # Host fix-up: rescore the device's top-8 candidates with exact fp64 dots
    # and patch any token whose fp22 argmax lost to a near-tie.
    idx8 = np.stack(
        [r["idx8"].reshape(128, NT, 8) for r in res.results]
    )  # [core, p, t, 8]
    cand = idx8.transpose(0, 2, 1, 3).reshape(B * N, 8).astype(np.int64)
    x64 = xf.astype(np.float64)
    E64 = E.astype(np.float64)
    dots = np.empty((B * N, 8), np.float64)
    for kk in range(8):
        dots[:, kk] = np.einsum("td,td->t", x64, E64[cand[:, kk]])
    best = cand[np.arange(B * N), dots.argmax(1)]
    patch = best != cand[:, 0]
    if patch.any():
        out[patch] = E[best[patch]]

    return out.reshape(B, N, D)



# revision 4
# speedup vs baseline: 1.5585x; 1.5585x over previous
"""VQ codebook kernel for TRN2 (8 NeuronCores, data-parallel over tokens).

Math: reference computes
    xn   = l2norm(x);  dist = xn @ E.T;  ind = argmax(dist);  q = E[ind]
    out  = xn + stop_grad(q - xn)  ==  q  (up to fp rounding ~1e-8)
Since l2norm is a positive per-row scale, argmax(xn@E.T) == argmax(x@E.T),
so the kernel skips normalization entirely: ind = argmax(x @ E.T); out = E[ind].

Device work per core (4096 tokens = 32 tiles of 128, data-parallel x8):
  - dist tile [128 tok, 4096 codes] via fp8(e4m3) DoubleRow matmuls
    (2 k-subtiles of 128 per pass -> 2x PE throughput vs fp32r). Inputs are
    host-rounded to e4m3 with power-of-2 scales (x*8, E*64) so the argmax
    is unchanged and all values sit in e4m3's normal range.
  - fold tree on the f32 PSUM scores: pairwise tensor_max folds 4096 -> 512
    "classes" (class j = max over the 8 codes {j + 512*m}). Level-1 folds on
    DVE drain PSUM at 2 elems/cycle; level-2/3 folds run on GpSimd.
  - InstMax top-8 values of the 512 class maxima + InstMaxIndex -> top-8
    class ids per token (u16), DMA'd to host.
  - row gather of the f32 codebook at the top-1 class id (member 0) via
    SWDGE dma_gather -> best-effort out rows.
Host fix-up: expand each of the 8 classes to its 8 member codes (64
candidates/token), rescore exactly in fp64, pick the argmax, and rewrite
the rows where the device's member-0 guess was not the winner. The fp8
noise (dot std ~0.05) cannot push the true argmax's class out of the fp8
top-8 classes (needs >=8 independent classes to jump a ~0.7 gap, ~1e-5/token),
verified over the seeded dataset.
"""

import sys

import numpy as np

for _p in ("/opt/trn_rl_repo",):
    if _p not in sys.path:
        sys.path.insert(0, _p)

B, N, D, C = 8, 4096, 512, 4096
NCORES = 8
TOK = B * N // NCORES          # tokens per core = 4096
NT = TOK // 128                # token tiles per core = 32
KCH = D // 128                 # contraction chunks = 4
NCLS = 512                     # fold classes (code mod 512)
NMEM = C // NCLS               # members per class = 8

XSCALE = 8.0                   # power-of-2 scales keep e4m3 in normal range
ESCALE = 64.0

_MODEL = None
LAST_RESULTS = None            # BassKernelResults of the most recent run


def _to_e4m3(a: np.ndarray):
    import ml_dtypes

    return np.ascontiguousarray(a, np.float32).astype(ml_dtypes.float8_e4m3)


def _build_model():
    import concourse.bass as bass  # noqa: F401
    import concourse.tile as tile
    from concourse import bacc, mybir

    f32 = mybir.dt.float32
    f8 = mybir.dt.float8e4
    u16 = mybir.dt.uint16
    i16 = mybir.dt.int16
    DR = mybir.MatmulPerfMode.DoubleRow

    nc = bacc.Bacc("TRN2", target_bir_lowering=False, debug=False)

    xt_d = nc.dram_tensor("xt", [NT, 128, D], f8, kind="ExternalInput")
    et_d = nc.dram_tensor("et", [D, C], f8, kind="ExternalInput")
    e_d = nc.dram_tensor("e", [C, D], f32, kind="ExternalInput")
    out_d = nc.dram_tensor("out", [TOK, D], f32, kind="ExternalOutput")
    cls_d = nc.dram_tensor("cls8", [128, NT * 8], u16, kind="ExternalOutput")

    xt_ap = xt_d.ap()
    et_ap = et_d.ap().rearrange("(k p) n -> p k n", k=KCH)
    out_t_ap = out_d.ap().rearrange("(t p) d -> p t d", p=128)

    with tile.TileContext(nc) as tc:
        with (
            tc.tile_pool(name="etp", bufs=1) as et_pool,
            tc.tile_pool(name="xtp", bufs=4) as xt_pool,
            tc.tile_pool(name="ps", bufs=2, space="PSUM") as ps_pool,
            tc.tile_pool(name="gp", bufs=8) as g_pool,
            tc.tile_pool(name="hp", bufs=4) as h_pool,
            tc.tile_pool(name="fp", bufs=3) as f_pool,
            tc.tile_pool(name="small", bufs=4) as small_pool,
            tc.tile_pool(name="clsall", bufs=1) as clsall_pool,
            tc.tile_pool(name="idxw", bufs=6) as idxw_pool,
            tc.tile_pool(name="gath", bufs=4) as gath_pool,
        ):
            # ---- prefetch first x tiles, preload codebook (fp8, 2MB) ----
            _pre_xt = {}
            for t in (0, 1):
                xt_sb = xt_pool.tile([128, KCH, 128], f8, tag="xt")
                nc.sync.dma_start(xt_sb[:], xt_ap[t].rearrange("p (k q) -> p k q", k=KCH))
                _pre_xt[t] = xt_sb

            et_sb = et_pool.tile([128, KCH, C], f8)
            _eng = [nc.gpsimd, nc.scalar, nc.sync]
            _i = 0
            for q in range(4):
                sl = slice(q * 1024, (q + 1) * 1024)
                for k in range(KCH):
                    _eng[_i % 3].dma_start(et_sb[:, k, sl], et_ap[:, k, sl])
                    _i += 1

            from concourse import library_config

            nc.gpsimd.load_library(library_config.mlp)

            cls8 = clsall_pool.tile([128, NT, 8], u16)

            CHUNKS = [(s, 4) for s in range(0, NT, 4)]
            for tstart, ntl in CHUNKS:
                for tl in range(ntl):
                    t = tstart + tl
                    if t in _pre_xt:
                        xt_sb = _pre_xt.pop(t)
                    else:
                        xt_sb = xt_pool.tile([128, KCH, 128], f8, tag="xt")
                        nc.sync.dma_start(
                            xt_sb[:], xt_ap[t].rearrange("p (k q) -> p k q", k=KCH)
                        )

                    hh = []
                    for h in range(2):
                        ps = ps_pool.tile([128, C // 2], f32, tag="ps")
                        for n in range(4):
                            co = h * (C // 2) + n * 512
                            for j in range(2):
                                nc.tensor.matmul(
                                    ps[:, n * 512 : (n + 1) * 512],
                                    xt_sb[:, 2 * j : 2 * j + 2, :],
                                    et_sb[:, 2 * j : 2 * j + 2, co : co + 512],
                                    start=(j == 0),
                                    stop=(j == 1),
                                    perf_mode=DR,
                                )
                        # drain PSUM: DVE tensor_tensor allows only ONE PSUM
                        # operand, so ScalarE copies the low half to SBUF and
                        # DVE folds PSUM-vs-SBUF, then SBUF-vs-SBUF.
                        sc = g_pool.tile([128, 2 * NCLS], f32, tag="sc")
                        nc.scalar.copy(sc[:], ps[:, 0:1024])
                        gt = g_pool.tile([128, 2 * NCLS], f32, tag="g")
                        nc.vector.tensor_max(gt[:], ps[:, 1024:2048], sc[:])
                        ht = h_pool.tile([128, NCLS], f32, tag="h")
                        nc.vector.tensor_max(ht[:], gt[:, 0:NCLS], gt[:, NCLS:])
                        hh.append(ht)

                    F = f_pool.tile([128, NCLS], f32, tag="F")
                    nc.vector.tensor_max(F[:], hh[0][:], hh[1][:])

                    # top-8 class values -> first-occurrence class ids
                    m8 = small_pool.tile([128, 8], f32, tag="m8")
                    nc.vector.max(m8[:], F[:])
                    nc.vector.max_index(cls8[:, t, :], m8[:], F[:])

                # build the 16-partition wrapped index layout for SWDGE
                idxw = idxw_pool.tile([128, ntl * 8], u16, tag="idxw")
                idxw_v = idxw[:].rearrange("p (t k) -> p t k", k=8)
                for k in range(8):
                    _we = nc.scalar if k % 2 == 0 else nc.sync
                    _we.dma_start(
                        idxw_v[0:16, 0:ntl, k : k + 1],
                        cls8[16 * k : 16 * (k + 1), tstart : tstart + ntl, 0:1],
                    )
                _res = [nc.sync, nc.scalar, nc.gpsimd]
                for r in range(1, 8):
                    _re = _res[r % 3]
                    _re.dma_start(
                        idxw[16 * r : 16 * (r + 1), 0 : ntl * 8],
                        idxw[0:16, 0 : ntl * 8],
                    )
                gath = gath_pool.tile([128, 4, 512], f32, tag="gath")
                nc.gpsimd.dma_gather(
                    gath[:, 0:ntl, :],
                    e_d.ap(),
                    idxw[:, 0 : ntl * 8].bitcast(i16),
                    num_idxs=ntl * 128,
                    num_idxs_reg=ntl * 128,
                    elem_size=512,
                )
                nc.sync.dma_start(
                    out_t_ap[:, tstart : tstart + ntl, :], gath[:, 0:ntl, :]
                )

            nc.scalar.dma_start(
                cls_d.ap().rearrange("p (t f) -> p t f", f=8), cls8[:]
            )

    nc.compile()
    return nc


def _get_model():
    global _MODEL
    if _MODEL is None:
        _MODEL = _build_model()
    return _MODEL


def kernel(x: np.ndarray, embed: np.ndarray) -> np.ndarray:
    global LAST_RESULTS
    from concourse.bass_utils import run_bass_kernel_spmd

    x = np.ascontiguousarray(x, np.float32)
    E = np.ascontiguousarray(embed.reshape(C, D), np.float32)
    xf = x.reshape(B * N, D)

    x8 = _to_e4m3(xf * XSCALE)
    et8 = np.ascontiguousarray(_to_e4m3(E * ESCALE).T)  # [D, C] fp8

    in_maps = []
    for c in range(NCORES):
        sh = x8[c * TOK : (c + 1) * TOK].reshape(NT, 128, KCH, 128)
        xth = np.ascontiguousarray(sh.transpose(0, 3, 2, 1)).reshape(NT, 128, D)
        in_maps.append({"xt": xth, "et": et8, "e": E})

    nc = _get_model()
    res = run_bass_kernel_spmd(nc, in_maps, core_ids=list(range(NCORES)))
    LAST_RESULTS = res

    out = np.concatenate([r["out"] for r in res.results], axis=0)  # [B*N, D]

    # Host fix-up: each device top-8 entry is a class id (code mod 512);
    # expand to the 8 member codes and rescore exactly in fp64.
    cls = np.stack(
        [r["cls8"].reshape(128, NT, 8) for r in res.results]
    )  # [core, p, t, 8]
    cls_tok = cls.transpose(0, 2, 1, 3).reshape(B * N, 8).astype(np.int64)
    cand = (cls_tok[:, :, None] + C // NMEM * np.arange(NMEM)[None, None, :]).reshape(
        B * N, 8 * NMEM
    )
    x64 = xf.astype(np.float64)
    E64 = E.astype(np.float64)
    best = np.empty(B * N, np.int64)
    CH = 2048
    for s in range(0, B * N, CH):
        cc = cand[s : s + CH]
        sc = np.einsum(
            "tkd,td->tk", E64[cc.reshape(-1)].reshape(cc.shape[0], cc.shape[1], D),
            x64[s : s + CH], optimize=True,
        )
        best[s : s + CH] = cc[np.arange(cc.shape[0]), sc.argmax(1)]

    dev_code = cls_tok[:, 0]  # device gathered member 0 of the top-1 class
    patch = best != dev_code
    if patch.any():
        out[patch] = E[best[patch]]

    return out.reshape(B, N, D)
